# revision 13
# baseline (speedup 1.0000x reference)
"""Trainium2 Bass kernel for an 8-head MHA layer (B=2, T=S=2048, D=512, HS=64).

Sharding: batch x head-pair. Core c handles batch c//4 and heads
(2*(c%4), 2*(c%4)+1). Each core computes its two heads' attention plus their
contribution to the output projection; the host sums the 4 partial outputs
per batch and adds the projection bias.

Device-side layout (everything transposed so all contractions sit on the
SBUF partition axis):
  - Q^T/K^T/V^T [D, T] fed from host, D-tile major [4, 128, 2048]
  - q_h^T/k_h^T [HS=64, T]   (per-head projections)
  - v_h        [S, HS] with a ones-column appended (row-sum trick)
  - logits^T   [keys, rows] per 128-key tile -> exp on ACT (no max
    subtraction needed: logits ~ N(0,1), fp32 exp is safe)
  - attn^T @ v via PSUM accumulation; partition 64 of the [65, rows]
    result accumulates the softmax denominators l
  - normalization: broadcast 1/l across partitions with a K=2 matmul
    against a 2x128 indicator, one DVE multiply, then the output
    projection with both heads stacked on the contraction axis
"""

import numpy as np

B, T, S, D = 2, 2048, 2048, 512
H, HS = 8, 64
N_CORES = 8
HEADS_PER_CORE = 2
R_CHUNK = 512          # query rows processed per attention pass
KT_PER_LG = 2          # key-tiles per logits PSUM tile (exp batch)

_PROG = None           # cached (nc, names) so repeat kernel() calls skip rebuild


def _dt():
    import concourse.mybir as mybir
    return mybir.dt


def _build_program():
    from contextlib import ExitStack
    import concourse.bass as bass
    import concourse.mybir as mybir
    from concourse import bacc
    from concourse.tile import TileContext

    dt = mybir.dt
    F32 = dt.float32
    IN_DT = dt.float32r     # qkv + projection weights (matmul operands)
    QK_DT = dt.float32r     # q^T/k^T tiles (logits matmul operands)
    ATTN_DT = dt.float32r   # attn + v tiles (attn@v operands)
    PROJ_DT = dt.float32r   # mh_norm lhsT + projection kernel

    AF = mybir.ActivationFunctionType
    nc = bacc.Bacc("TRN2", target_bir_lowering=False, debug=False,
                   num_devices=N_CORES)

    qt_d = nc.dram_tensor("qt", [4, 128, T], IN_DT, kind="ExternalInput")
    kt_d = nc.dram_tensor("kt", [4, 128, S], IN_DT, kind="ExternalInput")
    vt_d = nc.dram_tensor("vt", [4, 128, S], IN_DT, kind="ExternalInput")
    wq_d = nc.dram_tensor("wq", [128, 512], IN_DT, kind="ExternalInput")
    wk_d = nc.dram_tensor("wk", [128, 512], IN_DT, kind="ExternalInput")
    wv_d = nc.dram_tensor("wv", [128, 512], IN_DT, kind="ExternalInput")
    pk_d = nc.dram_tensor("pk", [128, 512], PROJ_DT, kind="ExternalInput")
    ind0_d = nc.dram_tensor("ind0", [1, 128], F32, kind="ExternalInput")
    ind1_d = nc.dram_tensor("ind1", [1, 128], F32, kind="ExternalInput")
    ones_d = nc.dram_tensor("ones", [128, S // 128, 1], ATTN_DT,
                            kind="ExternalInput")
    out_d = nc.dram_tensor("out", [T, D], F32, kind="ExternalOutput")

    n_kt = S // 128              # 16 key tiles
    n_rc = T // R_CHUNK          # 4 row chunks
    n_lg = n_kt // KT_PER_LG     # logits psum tiles per (R, h)
    V_STRIDE = 66                # 65 used cols (64 HS + ones) + 1 pad

    with ExitStack() as ctx:
        tc = ctx.enter_context(TileContext(nc))
        const = ctx.enter_context(tc.tile_pool(name="const", bufs=1))
        work = ctx.enter_context(tc.tile_pool(name="work", bufs=2))
        ps_lg = ctx.enter_context(tc.tile_pool(name="ps_lg", bufs=2, space="PSUM"))
        ps_mh = ctx.enter_context(tc.tile_pool(name="ps_mh", bufs=2, space="PSUM"))
        ps_mi = ctx.enter_context(tc.tile_pool(name="ps_mi", bufs=2, space="PSUM"))

        # ---- load inputs -------------------------------------------------
        qt = const.tile([128, 4 * T], IN_DT)
        kt = const.tile([128, 4 * S], IN_DT)
        vt = const.tile([128, 4 * S], IN_DT)
        for d in range(4):
            nc.sync.dma_start(qt[:, d * T:(d + 1) * T], qt_d[d])
            nc.sync.dma_start(kt[:, d * S:(d + 1) * S], kt_d[d])
            nc.sync.dma_start(vt[:, d * S:(d + 1) * S], vt_d[d])
        wq = const.tile([128, 512], IN_DT)
        nc.sync.dma_start(wq[:], wq_d[:])
        wk = const.tile([128, 512], IN_DT)
        nc.sync.dma_start(wk[:], wk_d[:])
        wv = const.tile([128, 512], IN_DT)
        nc.sync.dma_start(wv[:], wv_d[:])
        pk = const.tile([128, 512], PROJ_DT)
        nc.sync.dma_start(pk[:], pk_d[:])
        ind0 = const.tile([1, 128], F32)
        nc.sync.dma_start(ind0[:], ind0_d[:])
        ind1 = const.tile([1, 128], F32)
        nc.sync.dma_start(ind1[:], ind1_d[:])

        # ---- per-head q^T / k^T projections ------------------------------
        qh = const.tile([64, 2 * T], QK_DT)     # head-major [64, h*T + t]
        kh = const.tile([64, 2 * S], QK_DT)
        for h in range(HEADS_PER_CORE):
            for c in range(T // 512):
                pq = ps_mi.tile([64, 512], F32, tag="mi")
                for d in range(4):
                    nc.tensor.matmul(
                        pq[:], wq[:, (h * 4 + d) * 64:(h * 4 + d + 1) * 64],
                        qt[:, d * T + c * 512: d * T + (c + 1) * 512],
                        start=(d == 0), stop=(d == 3))
                nc.vector.tensor_copy(qh[:, h * T + c * 512: h * T + (c + 1) * 512], pq[:])
                pk_ps = ps_mi.tile([64, 512], F32, tag="mi")
                for d in range(4):
                    nc.tensor.matmul(
                        pk_ps[:], wk[:, (h * 4 + d) * 64:(h * 4 + d + 1) * 64],
                        kt[:, d * S + c * 512: d * S + (c + 1) * 512],
                        start=(d == 0), stop=(d == 3))
                nc.vector.tensor_copy(kh[:, h * S + c * 512: h * S + (c + 1) * 512], pk_ps[:])

        # ---- v projection (natural [S, HS] layout, both heads fused) -----
        vh = [const.tile([128, n_kt * V_STRIDE], ATTN_DT, tag=f"vh{h}",
                         name=f"vh{h}")
              for h in range(HEADS_PER_CORE)]
        for h in range(HEADS_PER_CORE):
            ones_view = vh[h].rearrange("p (s c) -> p s c", c=V_STRIDE)
            nc.sync.dma_start(ones_view[:, :, 64:65], ones_d[:])
        for st in range(n_kt):
            pv = ps_mi.tile([128, 128], F32, tag="mi")
            for d in range(4):
                nc.tensor.matmul(
                    pv[:], vt[:, d * S + st * 128: d * S + (st + 1) * 128],
                    wv[:, d * 128:(d + 1) * 128],
                    start=(d == 0), stop=(d == 3))
            for h in range(HEADS_PER_CORE):
                nc.vector.tensor_copy(
                    vh[h][:, st * V_STRIDE: st * V_STRIDE + 64],
                    pv[:, h * 64:(h + 1) * 64])

        # ---- attention + output projection per row chunk ----------------
        for rc in range(n_rc):
            r0 = rc * R_CHUNK
            lhsT = work.tile([128, R_CHUNK], PROJ_DT, tag="lhsT")
            recip = [work.tile([1, R_CHUNK], F32, tag=f"recip{h}",
                               name=f"recip{h}")
                     for h in range(HEADS_PER_CORE)]
            mh_ps = []
            for h in range(HEADS_PER_CORE):
                mh = ps_mh.tile([65, R_CHUNK], F32, tag="mh")
                mh_ps.append(mh)
                for lt in range(n_lg):
                    lg = ps_lg.tile([128, KT_PER_LG * R_CHUNK], F32, tag="lg")
                    attn = work.tile([128, KT_PER_LG * R_CHUNK], ATTN_DT, tag="attn")
                    for j in range(KT_PER_LG):
                        ktile = lt * KT_PER_LG + j
                        nc.tensor.matmul(
                            lg[:, j * R_CHUNK:(j + 1) * R_CHUNK],
                            kh[:, h * S + ktile * 128: h * S + (ktile + 1) * 128],
                            qh[:, h * T + r0: h * T + r0 + R_CHUNK],
                            start=True, stop=True)
                    # exp over the whole lg tile (scale folds in 1/sqrt(HS))
                    nc.scalar.activation(attn[:], lg[:], AF.Exp, scale=1.0 / np.sqrt(HS))
                    for j in range(KT_PER_LG):
                        ktile = lt * KT_PER_LG + j
                        nc.tensor.matmul(
                            mh[:], vh[h][:, ktile * V_STRIDE: ktile * V_STRIDE + 65],
                            attn[:, j * R_CHUNK:(j + 1) * R_CHUNK],
                            start=(ktile == 0), stop=(ktile == n_kt - 1))
                nc.vector.reciprocal(recip[h][:], mh[64:65, :])
            # broadcast 1/l_h across partitions 64h..64h+63 via K=1 matmuls
            bc = ps_mi.tile([128, R_CHUNK], F32, tag="mi")
            nc.tensor.matmul(bc[:], ind0[:], recip[0][:], start=True, stop=False)
            nc.tensor.matmul(bc[:], ind1[:], recip[1][:], start=False, stop=True)
            bcs = work.tile([128, R_CHUNK], F32, tag="bcs")
            nc.vector.tensor_copy(bcs[:], bc[:])
            nc.vector.tensor_mul(lhsT[0:64, :], mh_ps[0][0:64, :], bcs[0:64, :])
            nc.vector.tensor_mul(lhsT[64:128, :], mh_ps[1][0:64, :], bcs[64:128, :])
            for rt in range(R_CHUNK // 128):
                po = ps_mi.tile([128, 512], F32, tag="mi")
                nc.tensor.matmul(po[:], lhsT[:, rt * 128:(rt + 1) * 128], pk[:],
                                 start=True, stop=True)
                osb = work.tile([128, 512], F32, tag="osb")
                nc.vector.tensor_copy(osb[:], po[:])
                nc.sync.dma_start(out_d[r0 + rt * 128: r0 + (rt + 1) * 128, :], osb[:])

    nc.compile()
    return nc


def _shard_inputs(query, key, value, query_kernel, key_kernel, value_kernel,
                  projection_kernel):
    """Build the 8 per-core input maps (all host-side numpy)."""
    f32 = np.float32
    in_maps = []
    per_batch = {}
    for b in range(B):
        qt = np.ascontiguousarray(query[b].T.reshape(4, 128, T)).astype(f32)
        kt = np.ascontiguousarray(key[b].T.reshape(4, 128, S)).astype(f32)
        vt = np.ascontiguousarray(value[b].T.reshape(4, 128, S)).astype(f32)
        per_batch[b] = (qt, kt, vt)
    ind = np.kron(np.eye(2), np.ones((1, 64))).reshape(2, 128).astype(f32)
    ind0, ind1 = ind[0:1].copy(), ind[1:2].copy()
    for c in range(N_CORES):
        b, hp = c // 4, c % 4
        h0 = HEADS_PER_CORE * hp
        qk = query_kernel[h0:h0 + 2].reshape(2, 4, 128, 64)
        kk = key_kernel[h0:h0 + 2].reshape(2, 4, 128, 64)
        vk = value_kernel[h0:h0 + 2].reshape(2, 4, 128, 64)
        wq = np.ascontiguousarray(qk.transpose(2, 0, 1, 3).reshape(128, 512)).astype(f32)
        wk = np.ascontiguousarray(kk.transpose(2, 0, 1, 3).reshape(128, 512)).astype(f32)
        wv = np.ascontiguousarray(vk.transpose(2, 1, 0, 3).reshape(128, 512)).astype(f32)
        pk = np.ascontiguousarray(
            projection_kernel[h0:h0 + 2].reshape(128, 512)).astype(f32)
        qt, kt, vt = per_batch[b]
        in_maps.append(dict(qt=qt, kt=kt, vt=vt, wq=wq, wk=wk, wv=wv, pk=pk,
                            ind0=ind0, ind1=ind1,
                            ones=np.ones((128, S // 128, 1), f32)))
    return in_maps


def _run(in_maps, trace=False):
    global _PROG
    from concourse.bass_utils import run_bass_kernel_spmd
    if _PROG is None:
        _PROG = _build_program()
    return run_bass_kernel_spmd(_PROG, in_maps, list(range(N_CORES)), trace=trace)


def kernel(query, key, value, query_kernel, key_kernel, value_kernel,
           projection_kernel, projection_bias, _trace=False):
    query = np.asarray(query, np.float32)
    key = np.asarray(key, np.float32)
    value = np.asarray(value, np.float32)
    query_kernel = np.asarray(query_kernel, np.float32)
    key_kernel = np.asarray(key_kernel, np.float32)
    value_kernel = np.asarray(value_kernel, np.float32)
    projection_kernel = np.asarray(projection_kernel, np.float32)
    projection_bias = np.asarray(projection_bias, np.float32)

    in_maps = _shard_inputs(query, key, value, query_kernel, key_kernel,
                            value_kernel, projection_kernel)
    res = _run(in_maps, trace=_trace)
    out = np.zeros((B, T, D), np.float32)
    for c in range(N_CORES):
        out[c // 4] += res.results[c]["out"]
    out += projection_bias[None, None, :]
    if _trace:
        kernel.last_exec_time_ns = res.exec_time_ns
    return out


# revision 19
# speedup vs baseline: 1.2341x; 1.2341x over previous
"""Trainium2 Bass kernel for an 8-head MHA layer (B=2, T=S=2048, D=512, HS=64).

Sharding: batch x head-pair. Core c handles batch c//4 and heads
(2*(c%4), 2*(c%4)+1). Each core computes its two heads' attention plus their
contribution to the output projection; the host sums the 4 partial outputs
per batch and adds the projection bias.

Device-side layout (everything transposed so all contractions sit on the
SBUF partition axis):
  - Q^T/K^T/V^T [D, T] fed from host, D-tile major [4, 128, 2048]
  - q_h^T/k_h^T [HS=64, T]   (per-head projections)
  - v_h        [S, HS] with a ones-column appended (row-sum trick)
  - logits^T   [keys, rows] per 128-key tile -> exp on ACT (no max
    subtraction needed: logits ~ N(0,1), fp32 exp is safe)
  - attn^T @ v via PSUM accumulation; partition 64 of the [65, rows]
    result accumulates the softmax denominators l
  - normalization: broadcast 1/l across partitions with a K=2 matmul
    against a 2x128 indicator, one DVE multiply, then the output
    projection with both heads stacked on the contraction axis
"""

import numpy as np

B, T, S, D = 2, 2048, 2048, 512
H, HS = 8, 64
N_CORES = 8
HEADS_PER_CORE = 2
R_CHUNK = 512          # query rows processed per attention pass
KT_PER_LG = 2          # key-tiles per logits PSUM tile (exp batch)

_PROG = None           # cached (nc, names) so repeat kernel() calls skip rebuild

import os
MM_DTYPE = os.environ.get("MHA_MM_DTYPE", "bfloat16")  # matmul operand dtype


def _np_in_dtype():
    if MM_DTYPE == "bfloat16":
        import ml_dtypes
        return np.dtype(ml_dtypes.bfloat16)
    return np.dtype(np.float32)


def _build_program():
    from contextlib import ExitStack
    import concourse.bass as bass
    import concourse.mybir as mybir
    from concourse import bacc
    from concourse.tile import TileContext

    dt = mybir.dt
    F32 = dt.float32
    MM_DT = getattr(dt, MM_DTYPE)
    IN_DT = MM_DT     # qkv + projection weights (matmul operands)
    QK_DT = MM_DT     # q^T/k^T tiles (logits matmul operands)
    ATTN_DT = MM_DT   # attn + v tiles (attn@v operands)
    PROJ_DT = MM_DT   # mh_norm lhsT + projection kernel

    AF = mybir.ActivationFunctionType
    nc = bacc.Bacc("TRN2", target_bir_lowering=False, debug=False,
                   num_devices=N_CORES)

    qt_d = nc.dram_tensor("qt", [4, 128, T], IN_DT, kind="ExternalInput")
    kt_d = nc.dram_tensor("kt", [4, 128, S], IN_DT, kind="ExternalInput")
    vt_d = nc.dram_tensor("vt", [4, 128, S], IN_DT, kind="ExternalInput")
    wq_d = nc.dram_tensor("wq", [128, 512], IN_DT, kind="ExternalInput")
    wk_d = nc.dram_tensor("wk", [128, 512], IN_DT, kind="ExternalInput")
    wv_d = nc.dram_tensor("wv", [128, 512], IN_DT, kind="ExternalInput")
    pk_d = nc.dram_tensor("pk", [128, 512], PROJ_DT, kind="ExternalInput")
    ind0_d = nc.dram_tensor("ind0", [1, 128], F32, kind="ExternalInput")
    ind1_d = nc.dram_tensor("ind1", [1, 128], F32, kind="ExternalInput")
    ones_d = nc.dram_tensor("ones", [128, S // 128], ATTN_DT,
                            kind="ExternalInput")
    out_d = nc.dram_tensor("out", [T, D], F32, kind="ExternalOutput")

    n_kt = S // 128              # 16 key tiles
    n_rc = T // R_CHUNK          # 4 row chunks
    n_lg = n_kt // KT_PER_LG     # logits psum tiles per (R, h)
    V_STRIDE = 66                # 65 used cols (64 HS + ones) + 1 pad

    with ExitStack() as ctx:
        tc = ctx.enter_context(TileContext(nc))
        const = ctx.enter_context(tc.tile_pool(name="const", bufs=1))
        work = ctx.enter_context(tc.tile_pool(name="work", bufs=2))
        ps_lg = ctx.enter_context(tc.tile_pool(name="ps_lg", bufs=2, space="PSUM"))
        ps_mh = ctx.enter_context(tc.tile_pool(name="ps_mh", bufs=2, space="PSUM"))
        ps_mi = ctx.enter_context(tc.tile_pool(name="ps_mi", bufs=2, space="PSUM"))

        # ---- load inputs -------------------------------------------------
        qt = const.tile([128, 4 * T], IN_DT)
        kt = const.tile([128, 4 * S], IN_DT)
        vt = const.tile([128, 4 * S], IN_DT)
        for d in range(4):
            nc.sync.dma_start(qt[:, d * T:(d + 1) * T], qt_d[d])
            nc.sync.dma_start(kt[:, d * S:(d + 1) * S], kt_d[d])
            nc.sync.dma_start(vt[:, d * S:(d + 1) * S], vt_d[d])
        wq = const.tile([128, 512], IN_DT)
        nc.sync.dma_start(wq[:], wq_d[:])
        wk = const.tile([128, 512], IN_DT)
        nc.sync.dma_start(wk[:], wk_d[:])
        wv = const.tile([128, 512], IN_DT)
        nc.sync.dma_start(wv[:], wv_d[:])
        pk = const.tile([128, 512], PROJ_DT)
        nc.sync.dma_start(pk[:], pk_d[:])
        ind0 = const.tile([1, 128], F32)
        nc.sync.dma_start(ind0[:], ind0_d[:])
        ind1 = const.tile([1, 128], F32)
        nc.sync.dma_start(ind1[:], ind1_d[:])

        # ---- per-head q^T / k^T projections ------------------------------
        qh = const.tile([64, 2 * T], QK_DT)     # head-major [64, h*T + t]
        kh = const.tile([64, 2 * S], QK_DT)
        for h in range(HEADS_PER_CORE):
            for c in range(T // 512):
                pq = ps_mi.tile([64, 512], F32, tag="mi")
                for d in range(4):
                    nc.tensor.matmul(
                        pq[:], wq[:, (h * 4 + d) * 64:(h * 4 + d + 1) * 64],
                        qt[:, d * T + c * 512: d * T + (c + 1) * 512],
                        start=(d == 0), stop=(d == 3))
                nc.vector.tensor_copy(qh[:, h * T + c * 512: h * T + (c + 1) * 512], pq[:])
                pk_ps = ps_mi.tile([64, 512], F32, tag="mi")
                for d in range(4):
                    nc.tensor.matmul(
                        pk_ps[:], wk[:, (h * 4 + d) * 64:(h * 4 + d + 1) * 64],
                        kt[:, d * S + c * 512: d * S + (c + 1) * 512],
                        start=(d == 0), stop=(d == 3))
                nc.vector.tensor_copy(kh[:, h * S + c * 512: h * S + (c + 1) * 512], pk_ps[:])

        # ---- v projection (natural [S, HS] layout, both heads fused) -----
        vh = [const.tile([128, n_kt * V_STRIDE], ATTN_DT, tag=f"vh{h}",
                         name=f"vh{h}")
              for h in range(HEADS_PER_CORE)]
        ones_sb = const.tile([128, S // 128], ATTN_DT)
        nc.sync.dma_start(ones_sb[:], ones_d[:])
        for h in range(HEADS_PER_CORE):
            for st in range(n_kt):
                nc.vector.tensor_copy(
                    vh[h][:, st * V_STRIDE + 64: st * V_STRIDE + 65],
                    ones_sb[:, st:st + 1])
        for st in range(n_kt):
            pv = ps_mi.tile([128, 128], F32, tag="mi")
            for d in range(4):
                nc.tensor.matmul(
                    pv[:], vt[:, d * S + st * 128: d * S + (st + 1) * 128],
                    wv[:, d * 128:(d + 1) * 128],
                    start=(d == 0), stop=(d == 3))
            for h in range(HEADS_PER_CORE):
                nc.vector.tensor_copy(
                    vh[h][:, st * V_STRIDE: st * V_STRIDE + 64],
                    pv[:, h * 64:(h + 1) * 64])

        # ---- attention + output projection per row chunk ----------------
        for rc in range(n_rc):
            r0 = rc * R_CHUNK
            lhsT = work.tile([128, R_CHUNK], PROJ_DT, tag="lhsT")
            recip = [work.tile([1, R_CHUNK], F32, tag=f"recip{h}",
                               name=f"recip{h}")
                     for h in range(HEADS_PER_CORE)]
            mh_ps = []
            for h in range(HEADS_PER_CORE):
                mh = ps_mh.tile([65, R_CHUNK], F32, tag="mh")
                mh_ps.append(mh)
                for lt in range(n_lg):
                    lg = ps_lg.tile([128, KT_PER_LG * R_CHUNK], F32, tag="lg")
                    attn = work.tile([128, KT_PER_LG * R_CHUNK], ATTN_DT, tag="attn")
                    for j in range(KT_PER_LG):
                        ktile = lt * KT_PER_LG + j
                        nc.tensor.matmul(
                            lg[:, j * R_CHUNK:(j + 1) * R_CHUNK],
                            kh[:, h * S + ktile * 128: h * S + (ktile + 1) * 128],
                            qh[:, h * T + r0: h * T + r0 + R_CHUNK],
                            start=True, stop=True)
                    # exp over the whole lg tile (scale folds in 1/sqrt(HS))
                    nc.scalar.activation(attn[:], lg[:], AF.Exp, scale=1.0 / np.sqrt(HS))
                    for j in range(KT_PER_LG):
                        ktile = lt * KT_PER_LG + j
                        nc.tensor.matmul(
                            mh[:], vh[h][:, ktile * V_STRIDE: ktile * V_STRIDE + 65],
                            attn[:, j * R_CHUNK:(j + 1) * R_CHUNK],
                            start=(ktile == 0), stop=(ktile == n_kt - 1))
                nc.vector.reciprocal(recip[h][:], mh[64:65, :])
            # broadcast 1/l_h across partitions 64h..64h+63 via K=1 matmuls
            bc = ps_mi.tile([128, R_CHUNK], F32, tag="mi")
            nc.tensor.matmul(bc[:], ind0[:], recip[0][:], start=True, stop=False)
            nc.tensor.matmul(bc[:], ind1[:], recip[1][:], start=False, stop=True)
            bcs = work.tile([128, R_CHUNK], F32, tag="bcs")
            nc.vector.tensor_copy(bcs[:], bc[:])
            nc.vector.tensor_mul(lhsT[0:64, :], mh_ps[0][0:64, :], bcs[0:64, :])
            nc.vector.tensor_mul(lhsT[64:128, :], mh_ps[1][0:64, :], bcs[64:128, :])
            for rt in range(R_CHUNK // 128):
                po = ps_mi.tile([128, 512], F32, tag="mi")
                nc.tensor.matmul(po[:], lhsT[:, rt * 128:(rt + 1) * 128], pk[:],
                                 start=True, stop=True)
                osb = work.tile([128, 512], F32, tag="osb")
                nc.vector.tensor_copy(osb[:], po[:])
                nc.sync.dma_start(out_d[r0 + rt * 128: r0 + (rt + 1) * 128, :], osb[:])

    nc.compile()
    return nc


def _shard_inputs(query, key, value, query_kernel, key_kernel, value_kernel,
                  projection_kernel):
    """Build the 8 per-core input maps (all host-side numpy)."""
    f32 = np.float32
    mdt = _np_in_dtype()
    in_maps = []
    per_batch = {}
    for b in range(B):
        qt = np.ascontiguousarray(query[b].T.reshape(4, 128, T)).astype(mdt)
        kt = np.ascontiguousarray(key[b].T.reshape(4, 128, S)).astype(mdt)
        vt = np.ascontiguousarray(value[b].T.reshape(4, 128, S)).astype(mdt)
        per_batch[b] = (qt, kt, vt)
    ind = np.kron(np.eye(2), np.ones((1, 64))).reshape(2, 128).astype(f32)
    ind0, ind1 = ind[0:1].copy(), ind[1:2].copy()
    for c in range(N_CORES):
        b, hp = c // 4, c % 4
        h0 = HEADS_PER_CORE * hp
        qk = query_kernel[h0:h0 + 2].reshape(2, 4, 128, 64)
        kk = key_kernel[h0:h0 + 2].reshape(2, 4, 128, 64)
        vk = value_kernel[h0:h0 + 2].reshape(2, 4, 128, 64)
        wq = np.ascontiguousarray(qk.transpose(2, 0, 1, 3).reshape(128, 512)).astype(mdt)
        wk = np.ascontiguousarray(kk.transpose(2, 0, 1, 3).reshape(128, 512)).astype(mdt)
        wv = np.ascontiguousarray(vk.transpose(2, 1, 0, 3).reshape(128, 512)).astype(mdt)
        pk = np.ascontiguousarray(
            projection_kernel[h0:h0 + 2].reshape(128, 512)).astype(mdt)
        qt, kt, vt = per_batch[b]
        in_maps.append(dict(qt=qt, kt=kt, vt=vt, wq=wq, wk=wk, wv=wv, pk=pk,
                            ind0=ind0, ind1=ind1,
                            ones=np.ones((128, S // 128), mdt)))
    return in_maps


def _run(in_maps, trace=False):
    global _PROG
    from concourse.bass_utils import run_bass_kernel_spmd
    if _PROG is None:
        _PROG = _build_program()
    return run_bass_kernel_spmd(_PROG, in_maps, list(range(N_CORES)), trace=trace)


def kernel(query, key, value, query_kernel, key_kernel, value_kernel,
           projection_kernel, projection_bias, _trace=False):
    query = np.asarray(query, np.float32)
    key = np.asarray(key, np.float32)
    value = np.asarray(value, np.float32)
    query_kernel = np.asarray(query_kernel, np.float32)
    key_kernel = np.asarray(key_kernel, np.float32)
    value_kernel = np.asarray(value_kernel, np.float32)
    projection_kernel = np.asarray(projection_kernel, np.float32)
    projection_bias = np.asarray(projection_bias, np.float32)

    in_maps = _shard_inputs(query, key, value, query_kernel, key_kernel,
                            value_kernel, projection_kernel)
    res = _run(in_maps, trace=_trace)
    out = np.zeros((B, T, D), np.float32)
    for c in range(N_CORES):
        out[c // 4] += res.results[c]["out"]
    out += projection_bias[None, None, :]
    if _trace:
        kernel.last_exec_time_ns = res.exec_time_ns
    return out
